# revision 7
# baseline (speedup 1.0000x reference)
"""Distributed causal multi-head attention block for Trainium2 (8 NeuronCores).

Problem: B=4, S=2048, E=1024, H=16 heads, fp32.
    q/k/v = Linear(query/key/value); causal softmax attention; out = Linear(attn).

Sharding: DP=4 over batch x TP=2 over heads. Core c = 2*b + g handles batch b
with heads [8g, 8g+8). Per-core:
  - QKV projections for its 8 heads (512 dims)
  - causal attention for those heads over the full sequence, computed in the
    *transposed* orientation: scoresT[k, q] so softmax needs no transposes;
    the softmax denominator comes from an extra ones-column in the AV matmul,
    normalization via DVE reciprocal + rank-1 broadcast matmul.
  - pair AllGather of the local attn output (attnT layout [512, 2048]), split
    into two collectives (S-halves) to overlap with attention compute
  - out-proj computes this core's 512 *output columns* (host slices Wo per
    core), so the instruction graph is rank-symmetric (SPMD).

All matmuls run in float32r (tf32) — inputs are pre-rounded to tf32 on the
host; PE accumulation is fp32, so the only error is the tf32 input rounding
(~5e-4 relative).
"""
import sys

if "/opt/trn_rl_repo" not in sys.path:
    sys.path.insert(0, "/opt/trn_rl_repo")

import numpy as np

import concourse.bacc as bacc
import concourse.tile as tile
import concourse.mybir as mybir
import concourse.bass_utils as bass_utils

f32 = mybir.dt.float32
f32r = mybir.dt.float32r
Exp = mybir.ActivationFunctionType.Exp

N_CORES = 8
B, S, E = 4, 2048, 1024
H, D = 16, 64
HC = 512            # per-core head dims (8 heads x 64)
SCALE = D ** -0.5
SQ = 512            # q-tile width (columns of scoresT)
SK = 128            # k-chunk (partition rows of scoresT)
NQT = S // SQ       # 4 q-tiles
NE = E // 128       # 8 contraction chunks of the E dim


def tf32_round(x: np.ndarray) -> np.ndarray:
    u = np.ascontiguousarray(x, dtype=np.float32).view(np.uint32)
    u = (u + 0x0FFF + ((u >> 13) & 1)) & np.uint32(0xFFFFE000)
    return u.view(np.float32)


def build_nc():
    nc = bacc.Bacc("TRN2", target_bir_lowering=False, debug=False,
                   num_devices=N_CORES)

    xq = nc.declare_dram_parameter("xq", [E, S], f32, isOutput=False)
    xk = nc.declare_dram_parameter("xk", [E, S], f32, isOutput=False)
    xv = nc.declare_dram_parameter("xv", [E, S], f32, isOutput=False)
    wq = nc.declare_dram_parameter("wq", [E, HC], f32, isOutput=False)
    wk = nc.declare_dram_parameter("wk", [E, HC], f32, isOutput=False)
    wv = nc.declare_dram_parameter("wv", [E, HC], f32, isOutput=False)
    wo = nc.declare_dram_parameter("wo", [E, HC], f32, isOutput=False)
    bq = nc.declare_dram_parameter("bq", [1, HC], f32, isOutput=False)
    bk = nc.declare_dram_parameter("bk", [1, HC], f32, isOutput=False)
    bv = nc.declare_dram_parameter("bv", [1, HC], f32, isOutput=False)
    bo = nc.declare_dram_parameter("bo", [1, HC], f32, isOutput=False)
    masks = nc.declare_dram_parameter("masks", [128, 4, SQ], f32, isOutput=False)
    vones = nc.declare_dram_parameter("vones", [128, 16, 8], f32, isOutput=False)
    ones = nc.declare_dram_parameter("ones", [1, SQ], f32, isOutput=False)
    out = nc.declare_dram_parameter("out", [S, HC], f32, isOutput=True)

    # AllGather staging: my attnT [512, 2048] split into two S-halves.
    agin = [nc.dram_tensor(f"agin{i}", [HC, S // 2], f32r) for i in range(2)]
    agout = [nc.dram_tensor(f"agout{i}", [2, HC, S // 2], f32r) for i in range(2)]
    RG = [[0, 1], [2, 3], [4, 5], [6, 7]]

    with tile.TileContext(nc) as tc:
        with tc.tile_pool(name="persist", bufs=1) as pp:
            qT = pp.tile([128, 4, S], f32r)       # [p, m, s]: q-dim = m*128+p
            kT = pp.tile([128, 4, S], f32r)
            v4 = pp.tile([128, 16, 8, 65], f32r)  # [p, sc, h, j]: v row sc*128+p
            wo_t = pp.tile([128, NE, HC], f32r)
            masks_t = pp.tile([128, 4, SQ], f32)
            ones_t = pp.tile([1, SQ], f32r)
            bq_t = pp.tile([1, HC], f32r)
            bk_t = pp.tile([1, HC], f32r)
            bv_t = pp.tile([1, HC], f32r)
            bo_t = pp.tile([1, HC], f32r)

            nc.sync.dma_start(out=masks_t[:], in_=masks[:, :, :])
            nc.sync.dma_start(out=ones_t[:], in_=ones[:, :].bitcast(f32r))
            nc.sync.dma_start(out=bq_t[:], in_=bq[:, :].bitcast(f32r))
            nc.sync.dma_start(out=bk_t[:], in_=bk[:, :].bitcast(f32r))
            nc.sync.dma_start(out=bv_t[:], in_=bv[:, :].bitcast(f32r))
            nc.sync.dma_start(out=bo_t[:], in_=bo[:, :].bitcast(f32r))
            nc.sync.dma_start(
                out=wo_t[:],
                in_=wo.ap().rearrange("(c p) n -> p c n", p=128).bitcast(f32r))
            nc.sync.dma_start(out=v4[:, :, :, 64], in_=vones[:, :, :].bitcast(f32r))

            # ---------------- Phase 1: QKV projections ----------------
            with tc.tile_pool(name="wx", bufs=2) as wx, \
                 tc.tile_pool(name="psA", bufs=3, space="PSUM") as psA:

                def proj_qk(dst, w_dram, b_tile, x_dram):
                    w_t = wx.tile([128, NE, HC], f32r, tag="w")
                    nc.sync.dma_start(
                        out=w_t[:],
                        in_=w_dram.ap().rearrange("(c p) n -> p c n", p=128)
                        .bitcast(f32r))
                    for n in range(4):
                        xs = wx.tile([128, NE, SQ], f32r, tag="x")
                        nc.sync.dma_start(
                            out=xs[:],
                            in_=x_dram.ap()
                            .rearrange("(c p) s -> p c s", p=128)[:, :, n * SQ:(n + 1) * SQ]
                            .bitcast(f32r))
                        for m in range(4):
                            ps = psA.tile([128, SQ], f32, tag="pp")
                            for kc in range(NE):
                                nc.tensor.matmul(
                                    ps[:], w_t[:, kc, m * 128:(m + 1) * 128],
                                    xs[:, kc, :],
                                    start=(kc == 0), stop=False)
                            nc.tensor.matmul(
                                ps[:], b_tile[0:1, m * 128:(m + 1) * 128],
                                ones_t[:], start=False, stop=True)
                            nc.vector.tensor_copy(
                                dst[:, m, n * SQ:(n + 1) * SQ], ps[:])

                proj_qk(kT, wk, bk_t, xk)

                # V projection: v4[s, h, j] (S-major), bias along free dim
                wv_t = wx.tile([128, NE, HC], f32r, tag="w")
                nc.sync.dma_start(
                    out=wv_t[:],
                    in_=wv.ap().rearrange("(c p) n -> p c n", p=128).bitcast(f32r))
                for n in range(4):
                    xs = wx.tile([128, NE, SQ], f32r, tag="x")
                    nc.sync.dma_start(
                        out=xs[:],
                        in_=xv.ap()
                        .rearrange("(c p) s -> p c s", p=128)[:, :, n * SQ:(n + 1) * SQ]
                        .bitcast(f32r))
                    for mm in range(4):
                        sc = n * 4 + mm
                        ps = psA.tile([128, SQ], f32, tag="pp")
                        for kc in range(NE):
                            nc.tensor.matmul(
                                ps[:], xs[:, kc, mm * 128:(mm + 1) * 128],
                                wv_t[:, kc, :], start=(kc == 0), stop=False)
                        nc.tensor.matmul(ps[:], ones_t[0:1, 0:128], bv_t[:],
                                         start=False, stop=True)
                        nc.vector.tensor_copy(
                            v4[:, sc, :, 0:64],
                            ps[:].rearrange("p (h j) -> p h j", h=8))

                proj_qk(qT, wq, bq_t, xq)

            # ---------------- Phase 2: causal attention ----------------
            with tc.tile_pool(name="att", bufs=3) as att, \
                 tc.tile_pool(name="attr", bufs=2) as attr, \
                 tc.tile_pool(name="psS", bufs=3, space="PSUM") as psS, \
                 tc.tile_pool(name="psAV", bufs=2, space="PSUM") as psAV, \
                 tc.tile_pool(name="psB", bufs=2, space="PSUM") as psB:
                for qt in range(NQT):
                    for h in range(8):
                        m, po = h // 2, 64 * (h % 2)
                        pav = psAV.tile([65, SQ], f32, tag="av")
                        nkc = (qt + 1) * (SQ // SK)
                        for kc in range(nkc):
                            pscore = psS.tile([128, SQ], f32, tag="sc")
                            nc.tensor.matmul(
                                pscore[:],
                                kT[po:po + 64, m, kc * SK:(kc + 1) * SK],
                                qT[po:po + 64, m, qt * SQ:(qt + 1) * SQ],
                                start=True, stop=True)
                            r = kc - 4 * qt
                            pt = att.tile([128, SQ], f32r, tag="pt")
                            if 0 <= r <= 3:
                                praw = attr.tile([128, SQ], f32, tag="praw")
                                nc.scalar.activation(praw[:], pscore[:], Exp,
                                                     scale=SCALE)
                                nc.vector.tensor_tensor(
                                    pt[:], praw[:], masks_t[:, r, :],
                                    op=mybir.AluOpType.mult)
                            else:
                                nc.scalar.activation(pt[:], pscore[:], Exp,
                                                     scale=SCALE)
                            nc.tensor.matmul(pav[:], v4[:, kc, h, :], pt[:],
                                             start=(kc == 0), stop=(kc == nkc - 1))
                        rt = attr.tile([1, SQ], f32r, tag="rt")
                        with nc.allow_low_precision(reason="tf32 pipeline"):
                            nc.vector.reciprocal(rt[:], pav[64:65, :])
                        pb = psB.tile([64, SQ], f32, tag="bc")
                        nc.tensor.matmul(pb[:], ones_t[0:1, 0:64], rt[:],
                                         start=True, stop=True)
                        pbs = attr.tile([64, SQ], f32, tag="pbs")
                        nc.vector.tensor_copy(pbs[:], pb[:])
                        at = attr.tile([64, SQ], f32r, tag="at")
                        nc.vector.tensor_tensor(at[:], pav[0:64, :], pbs[:],
                                                op=mybir.AluOpType.mult)
                        nc.sync.dma_start(
                            out=agin[qt // 2][h * 64:(h + 1) * 64,
                                              (qt % 2) * SQ:(qt % 2 + 1) * SQ],
                            in_=at[:])
                    if qt % 2 == 1:
                        nc.gpsimd.collective_compute(
                            "AllGather", mybir.AluOpType.bypass,
                            replica_groups=RG,
                            ins=[agin[qt // 2].ap().opt()],
                            outs=[agout[qt // 2].ap().opt()])

            # ---------------- Phase 3: out projection ----------------
            with tc.tile_pool(name="op", bufs=4) as op, \
                 tc.tile_pool(name="opo", bufs=3) as opo, \
                 tc.tile_pool(name="psO", bufs=3, space="PSUM") as psO:
                for part in range(2):
                    for mm in range(8):
                        po_ = psO.tile([128, HC], f32, tag="op")
                        for kcg in range(NE):
                            lt = op.tile([128, 128], f32r, tag="lt")
                            nc.sync.dma_start(
                                out=lt[:],
                                in_=agout[part][kcg // 4,
                                                (kcg % 4) * 128:(kcg % 4 + 1) * 128,
                                                mm * 128:(mm + 1) * 128])
                            nc.tensor.matmul(po_[:], lt[:], wo_t[:, kcg, :],
                                             start=(kcg == 0), stop=False)
                        nc.tensor.matmul(po_[:], ones_t[0:1, 0:128], bo_t[:],
                                         start=False, stop=True)
                        ot = opo.tile([128, HC], f32, tag="ot")
                        nc.vector.tensor_copy(ot[:], po_[:])
                        nc.sync.dma_start(
                            out=out[part * 1024 + mm * 128:part * 1024 + (mm + 1) * 128, :],
                            in_=ot[:])

    nc.compile()
    return nc


_NC_CACHE = None


def _get_nc():
    global _NC_CACHE
    if _NC_CACHE is None:
        _NC_CACHE = build_nc()
    return _NC_CACHE


def _prepare_in_maps(query, key, value, Wq, bq, Wk, bk, Wv, bv, Wo, bo):
    query = np.asarray(query, dtype=np.float32)
    key = np.asarray(key, dtype=np.float32)
    value = np.asarray(value, dtype=np.float32)

    xqT = [tf32_round(np.ascontiguousarray(query[b].T)) for b in range(B)]
    xkT = [tf32_round(np.ascontiguousarray(key[b].T)) for b in range(B)]
    xvT = [tf32_round(np.ascontiguousarray(value[b].T)) for b in range(B)]

    wq_g, wk_g, wv_g, wo_g = [], [], [], []
    bq_g, bk_g, bv_g, bo_g = [], [], [], []
    for g in range(2):
        sl = slice(g * HC, (g + 1) * HC)
        wq_g.append(tf32_round(np.ascontiguousarray(np.asarray(Wq)[sl, :].T)))
        wk_g.append(tf32_round(np.ascontiguousarray(np.asarray(Wk)[sl, :].T)))
        wv_g.append(tf32_round(np.ascontiguousarray(np.asarray(Wv)[sl, :].T)))
        wo_g.append(tf32_round(np.ascontiguousarray(np.asarray(Wo)[sl, :].T)))
        bq_g.append(tf32_round(np.asarray(bq)[sl].reshape(1, HC)))
        bk_g.append(tf32_round(np.asarray(bk)[sl].reshape(1, HC)))
        bv_g.append(tf32_round(np.asarray(bv)[sl].reshape(1, HC)))
        bo_g.append(tf32_round(np.asarray(bo)[sl].reshape(1, HC)))

    p = np.arange(128)[:, None, None]
    r = np.arange(4)[None, :, None]
    qn = np.arange(SQ)[None, None, :]
    masks = ((p + r * 128) <= qn).astype(np.float32)
    vones = np.ones((128, 16, 8), dtype=np.float32)
    ones = np.ones((1, SQ), dtype=np.float32)

    in_maps = []
    for c in range(N_CORES):
        b, g = c // 2, c % 2
        in_maps.append({
            "xq": xqT[b], "xk": xkT[b], "xv": xvT[b],
            "wq": wq_g[g], "wk": wk_g[g], "wv": wv_g[g], "wo": wo_g[g],
            "bq": bq_g[g], "bk": bk_g[g], "bv": bv_g[g], "bo": bo_g[g],
            "masks": masks, "vones": vones, "ones": ones,
        })
    return in_maps


def run(trace=False, **inputs):
    in_maps = _prepare_in_maps(**inputs)
    nc = _get_nc()
    res = bass_utils.run_bass_kernel_spmd(
        nc, in_maps, core_ids=list(range(N_CORES)), trace=trace)
    full = np.empty((B, S, E), dtype=np.float32)
    for c in range(N_CORES):
        b, g = c // 2, c % 2
        full[b, :, g * HC:(g + 1) * HC] = res.results[c]["out"]
    return full, res


def kernel(**inputs) -> np.ndarray:
    full, _ = run(trace=False, **inputs)
    return full


def bench(n_iters=5, repeats=5, **inputs):
    """Estimate on-device NEFF time: chain n_iters executions with a tiny
    data dependency (no CSE, strict serialization), time with device-resident
    inputs, and report the marginal per-iteration wall time."""
    import time
    import jax
    from jax.sharding import Mesh, PartitionSpec
    from jax.experimental.shard_map import shard_map
    import concourse.bass2jax as bass2jax
    import concourse.mybir as mb

    nc = _get_nc()
    in_maps = _prepare_in_maps(**inputs)
    bass2jax.install_neuronx_cc_hook()

    partition_name = nc.partition_id_tensor.name if nc.partition_id_tensor else None
    in_names, out_names, out_avals = [], [], []
    for alloc in nc.m.functions[0].allocations:
        if not isinstance(alloc, mb.MemoryLocationSet):
            continue
        name = alloc.memorylocations[0].name
        if alloc.kind == "ExternalInput":
            if name != partition_name:
                in_names.append(name)
        elif alloc.kind == "ExternalOutput":
            out_names.append(name)
            out_avals.append(
                jax.core.ShapedArray(tuple(alloc.tensor_shape),
                                     mb.dt.np(alloc.dtype)))
    n_params = len(in_names)
    all_in_names = list(in_names) + list(out_names)
    if partition_name is not None:
        all_in_names.append(partition_name)
    ones_idx = in_names.index("ones")

    def _body(*args):
        operands = list(args)
        if partition_name is not None:
            operands.append(bass2jax.partition_id_tensor())
        outs = bass2jax._bass_exec_p.bind(
            *operands,
            out_avals=tuple(out_avals),
            in_names=tuple(all_in_names),
            out_names=tuple(out_names),
            lowering_input_output_aliases=(),
            sim_require_finite=True,
            sim_require_nnan=True,
            nc=nc)
        return tuple(outs)

    devices = jax.devices()[:N_CORES]
    mesh = Mesh(np.asarray(devices), ("core",))
    n_outs = len(out_names)
    in_specs = (PartitionSpec("core"),) * (n_params + n_outs)
    out_specs = (PartitionSpec("core"),) * n_outs

    per_core = [[np.asarray(m[name]) for name in in_names] for m in in_maps]
    concat_in = [np.concatenate([per_core[c][i] for c in range(N_CORES)], axis=0)
                 for i in range(n_params)]
    concat_zeros = [np.zeros((N_CORES * a.shape[0], *a.shape[1:]), a.dtype)
                    for a in out_avals]

    sharding = jax.sharding.NamedSharding(mesh, PartitionSpec("core"))
    dev_in = [jax.device_put(x, sharding) for x in concat_in + concat_zeros]

    fn = jax.jit(shard_map(_body, mesh=mesh, in_specs=in_specs,
                           out_specs=out_specs, check_rep=False),
                 keep_unused=True)
    jax.block_until_ready(fn(*dev_in))  # warm
    samples = []
    for _ in range(repeats * n_iters):
        t0 = time.perf_counter()
        jax.block_until_ready(fn(*dev_in))
        samples.append(time.perf_counter() - t0)

    # dispatch baseline: trivial single-DMA NEFF through the same path
    base = _bench_baseline(mesh)
    full = min(samples)
    return (full - base) * 1e9, {"full": full, "base": base,
                                 "samples": sorted(samples)[:5]}


_BASE_NC = None


def _bench_baseline(mesh):
    import time
    import jax
    from jax.sharding import PartitionSpec
    from jax.experimental.shard_map import shard_map
    import concourse.bass2jax as bass2jax

    global _BASE_NC
    if _BASE_NC is None:
        nc = bacc.Bacc("TRN2", target_bir_lowering=False, debug=False,
                       num_devices=N_CORES)
        one = nc.declare_dram_parameter("one", [1, SQ], f32, isOutput=False)
        outp = nc.declare_dram_parameter("out", [1, SQ], f32, isOutput=True)
        with tile.TileContext(nc) as tc:
            with tc.tile_pool(name="sb", bufs=1) as sb:
                t = sb.tile([1, SQ], f32)
                nc.sync.dma_start(out=t[:], in_=one[:, :])
                nc.sync.dma_start(out=outp[:, :], in_=t[:])
        nc.compile()
        _BASE_NC = nc
    nc = _BASE_NC

    partition_name = nc.partition_id_tensor.name if nc.partition_id_tensor else None
    in_names = ["one", "out"]
    if partition_name is not None:
        in_names.append(partition_name)
    out_avals = (jax.core.ShapedArray((1, SQ), np.float32),)

    def _body(*args):
        operands = list(args)
        if partition_name is not None:
            operands.append(bass2jax.partition_id_tensor())
        outs = bass2jax._bass_exec_p.bind(
            *operands, out_avals=out_avals, in_names=tuple(in_names),
            out_names=("out",), lowering_input_output_aliases=(),
            sim_require_finite=True, sim_require_nnan=True, nc=nc)
        return tuple(outs)

    sharding = jax.sharding.NamedSharding(mesh, PartitionSpec("core"))
    ones = jax.device_put(np.ones((N_CORES, SQ), np.float32), sharding)
    zeros = jax.device_put(np.zeros((N_CORES, SQ), np.float32), sharding)
    fn = jax.jit(shard_map(_body, mesh=mesh,
                           in_specs=(PartitionSpec("core"),) * 2,
                           out_specs=(PartitionSpec("core"),),
                           check_rep=False), keep_unused=True)
    jax.block_until_ready(fn(ones, zeros))
    best = float("inf")
    for _ in range(20):
        t0 = time.perf_counter()
        jax.block_until_ready(fn(ones, zeros))
        best = min(best, time.perf_counter() - t0)
    return best


# revision 22
# speedup vs baseline: 2.1637x; 2.1637x over previous
"""Distributed causal multi-head attention block for Trainium2 (8 NeuronCores).

Problem: B=4, S=2048, E=1024, H=16 heads, fp32.
    q/k/v = Linear(query/key/value); causal softmax attention; out = Linear(attn).

Sharding: DP=4 over batch x TP=2 over heads. Core c = 2*b + g handles batch b
with heads [8g, 8g+8). Per-core:
  - QKV projections for its 8 heads (512 dims)
  - causal attention for those heads over the full sequence, computed in the
    *transposed* orientation: scoresT[k, q] so softmax needs no transposes;
    the softmax denominator comes from an extra ones-column in the AV matmul,
    normalization via DVE reciprocal + rank-1 broadcast matmul.
  - pair AllGather of the local attn output (attnT layout [512, 2048]), split
    into two collectives (S-halves) to overlap with attention compute
  - out-proj computes this core's 512 *output columns* (host slices Wo per
    core), so the instruction graph is rank-symmetric (SPMD).

All matmuls run in float32r (tf32) — inputs are pre-rounded to tf32 on the
host; PE accumulation is fp32, so the only error is the tf32 input rounding
(~5e-4 relative).
"""
import sys

if "/opt/trn_rl_repo" not in sys.path:
    sys.path.insert(0, "/opt/trn_rl_repo")

import numpy as np

import concourse.bacc as bacc
import concourse.tile as tile
import concourse.mybir as mybir
import concourse.bass_utils as bass_utils

f32 = mybir.dt.float32
f32r = mybir.dt.float32r
Exp = mybir.ActivationFunctionType.Exp

N_CORES = 8
B, S, E = 4, 2048, 1024
H, D = 16, 64
HC = 512            # per-core head dims (8 heads x 64)
SCALE = D ** -0.5
SQ = 512            # q-tile width (columns of scoresT)
SK = 128            # k-chunk (partition rows of scoresT)
NQT = S // SQ       # 4 q-tiles
NE = E // 128       # 8 contraction chunks of the E dim


def tf32_round(x: np.ndarray) -> np.ndarray:
    u = np.ascontiguousarray(x, dtype=np.float32).view(np.uint32)
    u = (u + 0x0FFF + ((u >> 13) & 1)) & np.uint32(0xFFFFE000)
    return u.view(np.float32)


def build_nc(skip_cc=False, lag=2):
    nc = bacc.Bacc("TRN2", target_bir_lowering=False, debug=False,
                   num_devices=N_CORES)

    xq = nc.declare_dram_parameter("xq", [E, S], f32, isOutput=False)
    xk = nc.declare_dram_parameter("xk", [E, S], f32, isOutput=False)
    xv = nc.declare_dram_parameter("xv", [E, S], f32, isOutput=False)
    wq = nc.declare_dram_parameter("wq", [E, HC], f32, isOutput=False)
    wk = nc.declare_dram_parameter("wk", [E, HC], f32, isOutput=False)
    wv = nc.declare_dram_parameter("wv", [E, HC], f32, isOutput=False)
    wo = nc.declare_dram_parameter("wo", [E, HC], f32, isOutput=False)
    bq = nc.declare_dram_parameter("bq", [1, HC], f32, isOutput=False)
    bk = nc.declare_dram_parameter("bk", [1, HC], f32, isOutput=False)
    bv = nc.declare_dram_parameter("bv", [1, HC], f32, isOutput=False)
    bo = nc.declare_dram_parameter("bo", [1, HC], f32, isOutput=False)
    masks = nc.declare_dram_parameter("masks", [128, 4, SQ], f32, isOutput=False)
    vones = nc.declare_dram_parameter("vones", [128, 16, 8], f32, isOutput=False)
    ones = nc.declare_dram_parameter("ones", [1, SQ], f32, isOutput=False)
    out = nc.declare_dram_parameter("out", [S, HC], f32, isOutput=True)

    # AllGather staging: my attnT [512, 2048] split into two S-halves.
    agin = [nc.dram_tensor(f"agin{i}", [HC, S // 2], f32r) for i in range(2)]
    agout = [nc.dram_tensor(f"agout{i}", [2, HC, S // 2], f32r) for i in range(2)]
    RG = [[0, 1], [2, 3], [4, 5], [6, 7]]

    with tile.TileContext(nc) as tc:
        with tc.tile_pool(name="persist", bufs=1) as pp:
            qT = pp.tile([128, 4, S], f32r)       # [p, m, s]: q-dim = m*128+p
            kT = pp.tile([128, 4, S], f32r)
            v4 = pp.tile([128, 16, 8, 65], f32r)  # [p, sc, h, j]: v row sc*128+p
            wo_t = pp.tile([128, NE, HC], f32r)
            masks_t = pp.tile([128, 4, SQ], f32)
            ones_t = pp.tile([1, SQ], f32r)
            bq_t = pp.tile([1, HC], f32r)
            bk_t = pp.tile([1, HC], f32r)
            bv_t = pp.tile([1, HC], f32r)
            bo_t = pp.tile([1, HC], f32r)

            nc.sync.dma_start(out=ones_t[:], in_=ones[:, :].bitcast(f32r))
            nc.sync.dma_start(out=bq_t[:], in_=bq[:, :].bitcast(f32r))
            nc.sync.dma_start(out=bk_t[:], in_=bk[:, :].bitcast(f32r))
            nc.sync.dma_start(out=bv_t[:], in_=bv[:, :].bitcast(f32r))
            nc.sync.dma_start(out=bo_t[:], in_=bo[:, :].bitcast(f32r))

            # ---------------- Phase 1: QKV projections ----------------
            with tc.tile_pool(name="wx", bufs=2) as wx, \
                 tc.tile_pool(name="psA", bufs=3, space="PSUM") as psA:

                def proj_qk(dst, w_dram, b_tile, x_dram):
                    w_t = wx.tile([128, NE, HC], f32r, tag="w")
                    nc.sync.dma_start(
                        out=w_t[:],
                        in_=w_dram.ap().rearrange("(c p) n -> p c n", p=128)
                        .bitcast(f32r))
                    for n in range(4):
                        xs = wx.tile([128, NE, SQ], f32r, tag="x")
                        nc.sync.dma_start(
                            out=xs[:],
                            in_=x_dram.ap()
                            .rearrange("(c p) s -> p c s", p=128)[:, :, n * SQ:(n + 1) * SQ]
                            .bitcast(f32r))
                        for m in range(4):
                            ps = psA.tile([128, SQ], f32, tag="pp")
                            for kc in range(NE):
                                nc.tensor.matmul(
                                    ps[:], w_t[:, kc, m * 128:(m + 1) * 128],
                                    xs[:, kc, :],
                                    start=(kc == 0), stop=False)
                            nc.tensor.matmul(
                                ps[:], b_tile[0:1, m * 128:(m + 1) * 128],
                                ones_t[:], start=False, stop=True)
                            nc.vector.tensor_copy(
                                dst[:, m, n * SQ:(n + 1) * SQ], ps[:])

                proj_qk(kT, wk, bk_t, xk)

                # issue mid-priority loads now: needed for attention, after
                # the first projection's weights/slabs are already in flight
                nc.sync.dma_start(out=masks_t[:], in_=masks[:, :, :])
                nc.sync.dma_start(out=v4[:, :, :, 64],
                                  in_=vones[:, :, :].bitcast(f32r))

                # V projection: v4[s, h, j] (S-major), bias along free dim
                wv_t = wx.tile([128, NE, HC], f32r, tag="w")
                nc.sync.dma_start(
                    out=wv_t[:],
                    in_=wv.ap().rearrange("(c p) n -> p c n", p=128).bitcast(f32r))
                for n in range(4):
                    xs = wx.tile([128, NE, SQ], f32r, tag="x")
                    nc.sync.dma_start(
                        out=xs[:],
                        in_=xv.ap()
                        .rearrange("(c p) s -> p c s", p=128)[:, :, n * SQ:(n + 1) * SQ]
                        .bitcast(f32r))
                    for mm in range(4):
                        sc = n * 4 + mm
                        ps = psA.tile([128, SQ], f32, tag="pp")
                        for kc in range(NE):
                            nc.tensor.matmul(
                                ps[:], xs[:, kc, mm * 128:(mm + 1) * 128],
                                wv_t[:, kc, :], start=(kc == 0), stop=False)
                        nc.tensor.matmul(ps[:], ones_t[0:1, 0:128], bv_t[:],
                                         start=False, stop=True)
                        nc.vector.tensor_copy(
                            v4[:, sc, :, 0:64],
                            ps[:].rearrange("p (h j) -> p h j", h=8))

                proj_qk(qT, wq, bq_t, xq)

                # out-proj weights: issued now so the 4MB load streams in
                # during the attention phase
                nc.sync.dma_start(
                    out=wo_t[:],
                    in_=wo.ap().rearrange("(c p) n -> p c n", p=128).bitcast(f32r))

            # ---------------- Phase 2: causal attention ----------------
            with tc.tile_pool(name="att", bufs=lag + 4) as att, \
                 tc.tile_pool(name="attr", bufs=4) as attr, \
                 tc.tile_pool(name="psS", bufs=lag + 1, space="PSUM") as psS, \
                 tc.tile_pool(name="psAV", bufs=2, space="PSUM") as psAV, \
                 tc.tile_pool(name="psB", bufs=2, space="PSUM") as psB:
                pending_fin = None
                for qt in range(NQT):
                    for h in range(8):
                        m, po = h // 2, 64 * (h % 2)
                        pav = psAV.tile([65, SQ], f32, tag="av")
                        nkc = (qt + 1) * (SQ // SK)
                        pts = {}

                        def issue_score(kc, qt=qt, h=h, m=m, po=po, pts=pts):
                            pscore = psS.tile([128, SQ], f32, tag="sc")
                            nc.tensor.matmul(
                                pscore[:],
                                kT[po:po + 64, m, kc * SK:(kc + 1) * SK],
                                qT[po:po + 64, m, qt * SQ:(qt + 1) * SQ],
                                start=True, stop=True)
                            r = kc - 4 * qt
                            pt = att.tile([128, SQ], f32r, tag="pt")
                            if 0 <= r <= 3:
                                # columns < r*128 are fully masked: skip them
                                c0 = r * SK
                                praw = attr.tile([128, SQ], f32, tag="praw")
                                nc.scalar.activation(praw[:, c0:], pscore[:, c0:],
                                                     Exp, scale=SCALE)
                                nc.vector.tensor_tensor(
                                    pt[:, c0:], praw[:, c0:], masks_t[:, r, c0:],
                                    op=mybir.AluOpType.mult)
                                pts[kc] = (pt, c0)
                            else:
                                nc.scalar.activation(pt[:], pscore[:], Exp,
                                                     scale=SCALE)
                                pts[kc] = (pt, 0)

                        for kc in range(min(lag, nkc)):
                            issue_score(kc)
                        # finalize the previous head while this head's score
                        # pipeline fills, so TensorE never waits on the DVE
                        # reciprocal chain
                        if pending_fin is not None:
                            pending_fin()
                            pending_fin = None
                        for kc in range(nkc):
                            if kc + lag < nkc:
                                issue_score(kc + lag)
                            pt, c0 = pts.pop(kc)
                            # kc==0 always has c0==0, so start covers the
                            # whole [65, 512] accumulator
                            nc.tensor.matmul(pav[:, c0:], v4[:, kc, h, :],
                                             pt[:, c0:],
                                             start=(kc == 0), stop=(kc == nkc - 1))

                        def finalize(qt=qt, h=h, pav=pav):
                            rt = attr.tile([1, SQ], f32r, tag="rt")
                            with nc.allow_low_precision(reason="tf32 pipeline"):
                                nc.vector.reciprocal(rt[:], pav[64:65, :])
                            pb = psB.tile([64, SQ], f32, tag="bc")
                            nc.tensor.matmul(pb[:], ones_t[0:1, 0:64], rt[:],
                                             start=True, stop=True)
                            pbs = attr.tile([64, SQ], f32, tag="pbs")
                            nc.vector.tensor_copy(pbs[:], pb[:])
                            at = attr.tile([64, SQ], f32r, tag="at")
                            nc.vector.tensor_tensor(at[:], pav[0:64, :], pbs[:],
                                                    op=mybir.AluOpType.mult)
                            nc.sync.dma_start(
                                out=agin[qt // 2][h * 64:(h + 1) * 64,
                                                  (qt % 2) * SQ:(qt % 2 + 1) * SQ],
                                in_=at[:])

                        pending_fin = finalize
                    if qt % 2 == 1:
                        if pending_fin is not None:
                            pending_fin()
                            pending_fin = None
                        if not skip_cc:
                            nc.gpsimd.collective_compute(
                                "AllGather", mybir.AluOpType.bypass,
                                replica_groups=RG,
                                ins=[agin[qt // 2].ap().opt()],
                                outs=[agout[qt // 2].ap().opt()])

            # ---------------- Phase 3: out projection ----------------
            with tc.tile_pool(name="op", bufs=3) as op, \
                 tc.tile_pool(name="opo", bufs=3) as opo, \
                 tc.tile_pool(name="psO", bufs=3, space="PSUM") as psO:
                for part in range(2):
                    for mm in range(8):
                        # one batched load of all 8 lhsT K-chunks for this
                        # output row-tile
                        lt = op.tile([128, 2, 4, 128], f32r, tag="lt")
                        if skip_cc:
                            src = agin[part].ap().rearrange(
                                "(ic p) s -> p ic s", p=128)[:, :, mm * 128:(mm + 1) * 128]
                            nc.sync.dma_start(out=lt[:, 0, :, :], in_=src)
                            nc.sync.dma_start(out=lt[:, 1, :, :], in_=src)
                        else:
                            nc.sync.dma_start(
                                out=lt[:],
                                in_=agout[part].ap().rearrange(
                                    "j (ic p) s -> p j ic s",
                                    p=128)[:, :, :, mm * 128:(mm + 1) * 128])
                        po_ = psO.tile([128, HC], f32, tag="op")
                        for kcg in range(NE):
                            nc.tensor.matmul(po_[:], lt[:, kcg // 4, kcg % 4, :],
                                             wo_t[:, kcg, :],
                                             start=(kcg == 0), stop=False)
                        nc.tensor.matmul(po_[:], ones_t[0:1, 0:128], bo_t[:],
                                         start=False, stop=True)
                        ot = opo.tile([128, HC], f32, tag="ot")
                        nc.vector.tensor_copy(ot[:], po_[:])
                        nc.sync.dma_start(
                            out=out[part * 1024 + mm * 128:part * 1024 + (mm + 1) * 128, :],
                            in_=ot[:])

    nc.compile()
    return nc


_NC_CACHE = None


def _get_nc():
    global _NC_CACHE
    if _NC_CACHE is None:
        _NC_CACHE = build_nc()
    return _NC_CACHE


def _prepare_in_maps(query, key, value, Wq, bq, Wk, bk, Wv, bv, Wo, bo):
    query = np.asarray(query, dtype=np.float32)
    key = np.asarray(key, dtype=np.float32)
    value = np.asarray(value, dtype=np.float32)

    xqT = [tf32_round(np.ascontiguousarray(query[b].T)) for b in range(B)]
    xkT = [tf32_round(np.ascontiguousarray(key[b].T)) for b in range(B)]
    xvT = [tf32_round(np.ascontiguousarray(value[b].T)) for b in range(B)]

    wq_g, wk_g, wv_g, wo_g = [], [], [], []
    bq_g, bk_g, bv_g, bo_g = [], [], [], []
    for g in range(2):
        sl = slice(g * HC, (g + 1) * HC)
        wq_g.append(tf32_round(np.ascontiguousarray(np.asarray(Wq)[sl, :].T)))
        wk_g.append(tf32_round(np.ascontiguousarray(np.asarray(Wk)[sl, :].T)))
        wv_g.append(tf32_round(np.ascontiguousarray(np.asarray(Wv)[sl, :].T)))
        wo_g.append(tf32_round(np.ascontiguousarray(np.asarray(Wo)[sl, :].T)))
        bq_g.append(tf32_round(np.asarray(bq)[sl].reshape(1, HC)))
        bk_g.append(tf32_round(np.asarray(bk)[sl].reshape(1, HC)))
        bv_g.append(tf32_round(np.asarray(bv)[sl].reshape(1, HC)))
        bo_g.append(tf32_round(np.asarray(bo)[sl].reshape(1, HC)))

    p = np.arange(128)[:, None, None]
    r = np.arange(4)[None, :, None]
    qn = np.arange(SQ)[None, None, :]
    masks = ((p + r * 128) <= qn).astype(np.float32)
    vones = np.ones((128, 16, 8), dtype=np.float32)
    ones = np.ones((1, SQ), dtype=np.float32)

    in_maps = []
    for c in range(N_CORES):
        b, g = c // 2, c % 2
        in_maps.append({
            "xq": xqT[b], "xk": xkT[b], "xv": xvT[b],
            "wq": wq_g[g], "wk": wk_g[g], "wv": wv_g[g], "wo": wo_g[g],
            "bq": bq_g[g], "bk": bk_g[g], "bv": bv_g[g], "bo": bo_g[g],
            "masks": masks, "vones": vones, "ones": ones,
        })
    return in_maps


def run(trace=False, **inputs):
    in_maps = _prepare_in_maps(**inputs)
    nc = _get_nc()
    res = bass_utils.run_bass_kernel_spmd(
        nc, in_maps, core_ids=list(range(N_CORES)), trace=trace)
    full = np.empty((B, S, E), dtype=np.float32)
    for c in range(N_CORES):
        b, g = c // 2, c % 2
        full[b, :, g * HC:(g + 1) * HC] = res.results[c]["out"]
    return full, res


def kernel(**inputs) -> np.ndarray:
    full, _ = run(trace=False, **inputs)
    return full


def bench(n_iters=5, repeats=5, **inputs):
    """Estimate on-device NEFF time: chain n_iters executions with a tiny
    data dependency (no CSE, strict serialization), time with device-resident
    inputs, and report the marginal per-iteration wall time."""
    import time
    import jax
    from jax.sharding import Mesh, PartitionSpec
    from jax.experimental.shard_map import shard_map
    import concourse.bass2jax as bass2jax
    import concourse.mybir as mb

    nc = _get_nc()
    in_maps = _prepare_in_maps(**inputs)
    bass2jax.install_neuronx_cc_hook()

    partition_name = nc.partition_id_tensor.name if nc.partition_id_tensor else None
    in_names, out_names, out_avals = [], [], []
    for alloc in nc.m.functions[0].allocations:
        if not isinstance(alloc, mb.MemoryLocationSet):
            continue
        name = alloc.memorylocations[0].name
        if alloc.kind == "ExternalInput":
            if name != partition_name:
                in_names.append(name)
        elif alloc.kind == "ExternalOutput":
            out_names.append(name)
            out_avals.append(
                jax.core.ShapedArray(tuple(alloc.tensor_shape),
                                     mb.dt.np(alloc.dtype)))
    n_params = len(in_names)
    all_in_names = list(in_names) + list(out_names)
    if partition_name is not None:
        all_in_names.append(partition_name)
    ones_idx = in_names.index("ones")

    def _body(*args):
        operands = list(args)
        if partition_name is not None:
            operands.append(bass2jax.partition_id_tensor())
        outs = bass2jax._bass_exec_p.bind(
            *operands,
            out_avals=tuple(out_avals),
            in_names=tuple(all_in_names),
            out_names=tuple(out_names),
            lowering_input_output_aliases=(),
            sim_require_finite=True,
            sim_require_nnan=True,
            nc=nc)
        return tuple(outs)

    devices = jax.devices()[:N_CORES]
    mesh = Mesh(np.asarray(devices), ("core",))
    n_outs = len(out_names)
    in_specs = (PartitionSpec("core"),) * (n_params + n_outs)
    out_specs = (PartitionSpec("core"),) * n_outs

    per_core = [[np.asarray(m[name]) for name in in_names] for m in in_maps]
    concat_in = [np.concatenate([per_core[c][i] for c in range(N_CORES)], axis=0)
                 for i in range(n_params)]
    concat_zeros = [np.zeros((N_CORES * a.shape[0], *a.shape[1:]), a.dtype)
                    for a in out_avals]

    sharding = jax.sharding.NamedSharding(mesh, PartitionSpec("core"))
    dev_in = [jax.device_put(x, sharding) for x in concat_in + concat_zeros]

    fn = jax.jit(shard_map(_body, mesh=mesh, in_specs=in_specs,
                           out_specs=out_specs, check_rep=False),
                 keep_unused=True)
    jax.block_until_ready(fn(*dev_in))  # warm

    def run_m(m):
        best = float("inf")
        for _ in range(repeats):
            t0 = time.perf_counter()
            rs = [fn(*dev_in) for _ in range(m)]
            jax.block_until_ready(rs)
            del rs
            best = min(best, time.perf_counter() - t0)
        return best

    m_lo, m_hi = 2, 2 + n_iters
    t_lo, t_hi = run_m(m_lo), run_m(m_hi)
    marginal = (t_hi - t_lo) / (m_hi - m_lo)
    return marginal * 1e9, {"m_lo": (m_lo, t_lo), "m_hi": (m_hi, t_hi)}


_BASE_NC = None


def _bench_baseline(mesh):
    import time
    import jax
    from jax.sharding import PartitionSpec
    from jax.experimental.shard_map import shard_map
    import concourse.bass2jax as bass2jax

    global _BASE_NC
    if _BASE_NC is None:
        nc = bacc.Bacc("TRN2", target_bir_lowering=False, debug=False,
                       num_devices=N_CORES)
        one = nc.declare_dram_parameter("one", [1, SQ], f32, isOutput=False)
        outp = nc.declare_dram_parameter("out", [1, SQ], f32, isOutput=True)
        with tile.TileContext(nc) as tc:
            with tc.tile_pool(name="sb", bufs=1) as sb:
                t = sb.tile([1, SQ], f32)
                nc.sync.dma_start(out=t[:], in_=one[:, :])
                nc.sync.dma_start(out=outp[:, :], in_=t[:])
        nc.compile()
        _BASE_NC = nc
    nc = _BASE_NC

    partition_name = nc.partition_id_tensor.name if nc.partition_id_tensor else None
    in_names = ["one", "out"]
    if partition_name is not None:
        in_names.append(partition_name)
    out_avals = (jax.core.ShapedArray((1, SQ), np.float32),)

    def _body(*args):
        operands = list(args)
        if partition_name is not None:
            operands.append(bass2jax.partition_id_tensor())
        outs = bass2jax._bass_exec_p.bind(
            *operands, out_avals=out_avals, in_names=tuple(in_names),
            out_names=("out",), lowering_input_output_aliases=(),
            sim_require_finite=True, sim_require_nnan=True, nc=nc)
        return tuple(outs)

    sharding = jax.sharding.NamedSharding(mesh, PartitionSpec("core"))
    ones = jax.device_put(np.ones((N_CORES, SQ), np.float32), sharding)
    zeros = jax.device_put(np.zeros((N_CORES, SQ), np.float32), sharding)
    fn = jax.jit(shard_map(_body, mesh=mesh,
                           in_specs=(PartitionSpec("core"),) * 2,
                           out_specs=(PartitionSpec("core"),),
                           check_rep=False), keep_unused=True)
    jax.block_until_ready(fn(ones, zeros))
    best = float("inf")
    for _ in range(20):
        t0 = time.perf_counter()
        jax.block_until_ready(fn(ones, zeros))
        best = min(best, time.perf_counter() - t0)
    return best
